# revision 49
# baseline (speedup 1.0000x reference)
"""Trainium2 Bass kernel for nn_Kmeans (vq_codebook bucket assignment).

Reference computation:
    xn = normalize(x, dim=-1)                      # [b, l, d]
    dists = einsum('bhld,hcd->bhlc', xn, means)    # [b, h, l, c]
    buckets = argmax(dists, -1) + h*c              # [b, h*l]

argmax over c is invariant to the positive per-row scale 1/||x||, so the
normalization is skipped; we compute argmax_c(x @ means[h].T) in f32r.

Per 128-token row-tile the 512-wide argmax is computed in supergroups of
8 tiles (= 2 psum groups of 4):

  PE   : 8x f32r matmul -> psum [128, 2x(4x512)] fp32            (262/tile)
  ACT  : 2 batched converts psum -> SBUF fp16 d16 [128, 8, 512]  (473/tile)
  DVE  : pairwise TT-max fold tree at the fp16 2x rate
         d16 halves -> t1 [128,8,256] -> ... -> t4 [128,8,32],
         tail tensor_reduce -> m16 [128,8] fp16 (exact d16 values)
  DVE  : locate: per-tile max_index over d16 [128, 512] finds the first
         position of the tile's max = the full argmax           (~593/tile)

(Counts/locates on GPSIMD and 4x-mode tensor_scalar accum_out were tried
and rejected: walrus' engine check forbids tensor_tensor / STT / accum
ops on Pool, and the plain tensor_scalar accum_out silently returns zero
on hardware — only the scalar_tensor_tensor accum flavor works.)

Host decode: c = idx[u*8].  max_index's first-occurrence semantics
reproduces the reference argmax tie convention exactly; residual
mismatches are fp16-rounding order flips only.

Sharding: 16 (b, h) pairs across 8 cores, 2 pairs per core (one b, two h
per core); inputs pre-transposed host-side so device DMAs are contiguous:
    xm = [means[h0].T | x[b].T | means[h1].T]   # [64, 512+4096+512]
The x part streams in pieces whose first-consuming matmuls are never
psum-group-first, so every matmul keeps a single sync-wait.

walrus codegen accepts only ONE sync-wait per compute instruction; Tile
emits one wait per cross-engine dependency.  _fix_wait_limits keeps only
the wait whose producer is latest in the per-supergroup dependency chain
DMA -> PE -> ACT -> DVE -> Pool (each kept wait implies the dropped ones).
"""

import numpy as np

B, L, D = 4, 4096, 64
H, C = 4, 512
HC = C // 2  # 256
N_CORES = 8
PAIRS_PER_CORE = (B * H) // N_CORES  # 2
LTILE = 128
NT = L // LTILE  # 32
NTILES = PAIRS_PER_CORE * NT  # 64
GRP = 4                       # tiles per psum group (one batched convert)
SG = 2 * GRP                  # tiles per supergroup (one fold tree)
NSG = NTILES // SG            # 8

# x-piece boundaries (x tile indices).  Each piece's first-consuming matmul
# must not be the first matmul of a psum group (group-first matmuls carry the
# psum-WAR wait), so that the piece's DMA wait is that matmul's only wait.
X_PIECES = [(1, 5), (5, 14), (14, NT)]
# supergroups split into two half-trees at the pipeline ends to shorten the
# fill/drain chains
SPLIT_TREE_SGS = (0, NSG - 1)

# tile-slot map: slot -> (pair, x-tile).  Default pair-major order, with one
# swap: slot 7 runs pair-1 tile 0 so the means[h1] DMA's first-consuming
# matmul is a non-group-first slot (supergroup 0, psum group 1, matmul #4);
# pair-0 tile 7 runs in its place at slot 32.
SLOT_MAP = [divmod(u, NT) for u in range(NTILES)]
SLOT_MAP[7], SLOT_MAP[32] = SLOT_MAP[32], SLOT_MAP[7]

_CACHE = {}


def _build_nc():
    import concourse.bass as bass
    import concourse.tile as tile
    import concourse.mybir as mybir

    f32 = mybir.dt.float32
    f32r = mybir.dt.float32r
    f16 = mybir.dt.float16
    u16 = mybir.dt.uint16
    alu = mybir.AluOpType
    nc = bass.Bass()

    # host-side layout: xm = [means[h0].T | x[b].T | means[h1].T]
    ncol = 2 * C + L
    xm = nc.dram_tensor("xm", [D, ncol], f32r, kind="ExternalInput")
    outI = nc.dram_tensor("idx", [LTILE, 8 * NTILES], u16, kind="ExternalOutput")

    with tile.TileContext(nc) as tc:
        with (
            tc.tile_pool(name="xp", bufs=1) as xp,
            tc.tile_pool(name="pp", bufs=2, space="PSUM") as pp,
            # d16 buffers never recycle (bufs=NSG), so the ACT converts
            # carry no cross-engine WAR waits (their single-wait budget
            # goes to the PE psum RAW dependency)
            tc.tile_pool(name="dp", bufs=NSG) as dp,    # d16 supergroup bufs
            tc.tile_pool(name="t1p", bufs=2) as t1p,    # t1 (DVE-only)
            tc.tile_pool(name="t2p", bufs=2) as t2p,    # t2 (DVE-only)
            tc.tile_pool(name="tp", bufs=2) as tp,      # t3/t4 scratch
            tc.tile_pool(name="mp", bufs=4) as mp,      # m16p (DVE-only)
            tc.tile_pool(name="op", bufs=1) as op,
        ):
            sb = xp.tile([D, ncol], f32r, tag="A")
            # means[h0] + x tile 0 in one contiguous DMA (unblocks the first
            # matmuls with a single wait), then the x stream interleaved
            # with means[h1] (first consumed by slot 7, ~4.5us in)
            nc.sync.dma_start(sb[:, 0 : C + LTILE], xm[:, 0 : C + LTILE])
            lo, hi = X_PIECES[0]
            nc.sync.dma_start(
                sb[:, C + lo * LTILE : C + hi * LTILE],
                xm[:, C + lo * LTILE : C + hi * LTILE],
            )
            nc.sync.dma_start(sb[:, C + L :], xm[:, C + L :])
            for lo, hi in X_PIECES[1:]:
                nc.sync.dma_start(
                    sb[:, C + lo * LTILE : C + hi * LTILE],
                    xm[:, C + lo * LTILE : C + hi * LTILE],
                )

            def x_tile(t):
                c0 = C + t * LTILE
                return sb[:, c0 : c0 + LTILE]

            def m_ap(p):
                c0 = 0 if p == 0 else C + L
                return sb[:, c0 : c0 + C]

            idxall = op.tile([LTILE, 8 * NTILES], u16, tag="idxall")

            def fold_l1(t1, d16, lo, hi):
                """L1 fold of supergroup tile slots [lo, hi) into t1 slices;
                runs right after the corresponding convert half."""
                d = d16[:, lo:hi, :]
                nc.vector.tensor_tensor(
                    t1[:, lo:hi, :], d[:, :, 0:HC], d[:, :, HC:C], op=alu.max
                )

            def fold_rest(t1, t2, lo, hi, m16p):
                """tree levels 2..4 + tail reduce over t1 slots [lo, hi),
                writing per-tile maxes into m16p/m32 slices [lo, hi)."""
                n = hi - lo
                t1 = t1[:, lo:hi, :]
                nc.vector.tensor_tensor(
                    t2[:, lo:hi, :], t1[:, :, 0:128], t1[:, :, 128:256],
                    op=alu.max,
                )
                t3 = tp.tile([LTILE, n, 64], f16, tag="t3")
                nc.vector.tensor_tensor(
                    t3[:], t2[:, lo:hi, 0:64], t2[:, lo:hi, 64:128], op=alu.max
                )
                t4 = tp.tile([LTILE, n, 32], f16, tag="t4")
                nc.vector.tensor_tensor(
                    t4[:], t3[:, :, 0:32], t3[:, :, 32:64], op=alu.max
                )
                nc.vector.tensor_reduce(
                    m16p[:, lo:hi], t4[:], axis=mybir.AxisListType.X, op=alu.max
                )

            def locate(s, d16, m16p, lo, hi):
                """max_index per tile over its own d16 row [128, 512]: the
                first position of the tile's max = the full 9-bit argmax.
                Single-tile windows cannot collide with other tiles' values;
                in_max = m16p[:, i:i+8], whose entry 0 is this tile's max
                (later entries cannot steal the first match)."""
                for i in range(lo, hi):
                    nc.vector.max_index(
                        idxall[:, (s * SG + i) * 8 : (s * SG + i + 1) * 8],
                        m16p[:, i : i + 8], d16[:, i, :],
                    )

            for s in range(NSG):
                d16 = dp.tile([LTILE, SG, C], f16, tag="d16")
                t1 = t1p.tile([LTILE, SG, HC], f16, tag="t1")
                t2 = t2p.tile([LTILE, SG, 128], f16, tag="t2")
                m16p = mp.tile([LTILE, SG + 7], f16, tag="m16p")
                if s in SPLIT_TREE_SGS:
                    nc.vector.memset(m16p[:, GRP:], 30000.0)
                else:
                    nc.vector.memset(m16p[:, SG:], 30000.0)
                for half in range(2):
                    ps = pp.tile([LTILE, GRP * C], f32, tag="ps")
                    for i in range(GRP):
                        u = s * SG + half * GRP + i
                        p, t = SLOT_MAP[u]
                        nc.tensor.matmul(
                            ps[:, i * C : (i + 1) * C], x_tile(t), m_ap(p),
                            start=True, stop=True,
                        )
                    nc.scalar.copy(
                        d16[:, half * GRP : (half + 1) * GRP, :], ps[:]
                    )
                    lo, hi = half * GRP, (half + 1) * GRP
                    fold_l1(t1, d16, lo, hi)
                    if s in SPLIT_TREE_SGS:
                        fold_rest(t1, t2, lo, hi, m16p)
                        locate(s, d16, m16p, lo, hi)
                if s not in SPLIT_TREE_SGS:
                    fold_rest(t1, t2, 0, SG, m16p)
                    locate(s, d16, m16p, 0, SG)
            nc.sync.dma_start(outI[:], idxall[:])
    _fix_wait_limits(nc)
    return nc


def _fix_wait_limits(nc):
    """walrus codegen accepts only ONE sync-wait on compute/drain ISA
    structs; Tile emits one wait per cross-engine dependency.  Each
    multi-wait here forms a dependency chain DMA -> PE -> ACT -> DVE ->
    Pool whose final link implies the rest, so keep only the
    latest-producer wait."""
    import concourse.mybir as mybir

    flat = [i for f in nc.m.functions for blk in f.blocks for i in blk.instructions]

    sems = {}
    for inst in flat:
        si = inst.sync_info
        if si is None:
            continue
        nm = type(inst).__name__
        eng = getattr(inst, "engine", None)
        if nm == "InstDMACopy":
            rank = 0
        elif eng == mybir.EngineType.PE:
            rank = 1
        elif eng == mybir.EngineType.Activation:
            rank = 2
        elif eng == mybir.EngineType.DVE:
            rank = 3
        elif eng == mybir.EngineType.Pool:
            rank = 4
        else:
            rank = None
        if rank is not None:
            for u in si.on_update:
                sems[u.ant_name] = rank

    last_dma_sem = None
    for inst in flat:
        if type(inst).__name__ == "InstDMACopy" and inst.sync_info:
            for u in inst.sync_info.on_update:
                last_dma_sem = u.ant_name

    for inst in flat:
        nm = type(inst).__name__
        si = inst.sync_info
        if si is None or len(si.on_wait) <= 1:
            continue
        if nm == "InstDrain":
            keep = [w for w in si.on_wait if w.ant_name == last_dma_sem]
            assert len(keep) == 1, [str(w) for w in si.on_wait]
            inst.sync_info = mybir.SyncInfo(on_wait=keep, on_update=list(si.on_update))
        elif nm == "InstDMACopy":
            # output DMA: its engine-sem wait (single producer engine)
            # transitively implies every input DMA has landed
            keep = [w for w in si.on_wait if sems.get(w.ant_name, 0) >= 1]
            assert len(keep) == 1, [str(w) for w in si.on_wait]
            inst.sync_info = mybir.SyncInfo(on_wait=keep, on_update=list(si.on_update))
        else:
            # waits on the instruction's own engine's sems are implied by
            # engine program order; never pick them as the kept wait
            self_rank = {
                mybir.EngineType.PE: 1,
                mybir.EngineType.Activation: 2,
                mybir.EngineType.DVE: 3,
                mybir.EngineType.Pool: 4,
            }.get(getattr(inst, "engine", None))
            cands = [
                w for w in si.on_wait
                if sems.get(w.ant_name, -1) >= 1
                and sems.get(w.ant_name) != self_rank
            ]
            assert cands, [str(w) for w in si.on_wait]
            best = max(sems[w.ant_name] for w in cands)
            keep = [w for w in cands if sems[w.ant_name] == best]
            keep = [max(keep, key=lambda w: w.wait_value)]
            inst.sync_info = mybir.SyncInfo(on_wait=keep, on_update=list(si.on_update))
    return nc


def _decode(idx):
    """idx: [128, 8*NTILES] u16 max_index outputs, one 8-slot group per
    tile (slot 0 is the tile's own max).  Returns [128, NTILES] argmax
    indices in 0..511."""
    out = np.empty((idx.shape[0], NTILES), np.int32)
    for u in range(NTILES):
        out[:, u] = np.clip(idx[:, u * 8].astype(np.int32), 0, C - 1)
    return out


def kernel(x: np.ndarray, means: np.ndarray) -> np.ndarray:
    from concourse.bass_utils import run_bass_kernel_spmd

    x = np.ascontiguousarray(np.asarray(x, dtype=np.float32))
    means = np.ascontiguousarray(np.asarray(means, dtype=np.float32))
    assert x.shape == (B, L, D) and means.shape == (H, C, D)

    if "nc" not in _CACHE:
        _CACHE["nc"] = _build_nc()
    nc = _CACHE["nc"]

    mTfull = means.transpose(0, 2, 1)  # [H, D, C]
    in_maps = []
    for core in range(N_CORES):
        pairs = [core * PAIRS_PER_CORE + i for i in range(PAIRS_PER_CORE)]
        b = pairs[0] // H
        assert all(p // H == b for p in pairs)
        hs = [p % H for p in pairs]
        xmv = np.concatenate([mTfull[hs[0]], x[b].T, mTfull[hs[1]]], axis=1)
        in_maps.append({"xm": np.ascontiguousarray(xmv)})

    res = run_bass_kernel_spmd(
        nc,
        in_maps,
        core_ids=list(range(N_CORES)),
        trace=bool(_CACHE.get("trace", False)),
        **_CACHE.get("run_kwargs", {}),
    )
    _CACHE["last_result"] = res

    out = np.empty((B, H, L), dtype=np.int32)
    for core in range(N_CORES):
        idx = _decode(res.results[core]["idx"])
        for u in range(NTILES):
            p, t = SLOT_MAP[u]
            gp = core * PAIRS_PER_CORE + p
            b, h = gp // H, gp % H
            out[b, h, t * LTILE : (t + 1) * LTILE] = idx[:, u] + h * C
    return out.reshape(B, H * L)
